# revision 1
# baseline (speedup 1.0000x reference)
"""Multi-head attention (B=2, S=2048, D=1024, H=16, causal) on 8 TRN2 NeuronCores.

Sharding: core c -> (batch b = c//4, head-group hg = c%4). Each core:
  - projects its batch's query/key/value against a 256-row slice of Wq/Wk/Wv
    (4 heads of 64 dims),
  - runs causal attention for those 4 heads (scores computed transposed,
    exp on ACT with fused 1/8 scale, row-sums via a ones-column in V),
  - multiplies by the matching 256-column slice of Wo -> partial [2048, 1024].
Host sums the 4 partials per batch (the tensor-parallel all-reduce) and stacks.

Layout: operands are fed to the device pre-transposed ([din, tok] / [din, dout])
because the TensorE contracts over the partition dim and fp32 DMA-transpose is
not supported on TRN2.

Precision: matmuls run in float32r (TRN2's full-rate fp32 mode: inputs rounded
to 11 mantissa bits, fp32 accumulate in PSUM). Plain fp32 matmuls run at 1/4
rate. Inputs are pre-rounded on the host (round-half-up at bit 12, matching
hardware), which the BIR verifier accepts for direct DMA->matmul use.
"""

import sys

for _p in ("/opt/trn_rl_repo", "/root/.axon_site/_ro/trn_rl_repo"):
    if _p not in sys.path:
        sys.path.append(_p)

import numpy as np

import concourse.bacc as bacc
import concourse.tile as tile
import concourse.mybir as mybir
from concourse.bass import MemorySpace
from concourse.bass_utils import run_bass_kernel_spmd

f32 = mybir.dt.float32
f32r = mybir.dt.float32r
Exp = mybir.ActivationFunctionType.Exp

B, S, D, H = 2, 2048, 1024, 16
HD = 64            # head dim
NH = 4             # heads per core
DO = NH * HD       # 256 projection out-dims per core
NCORES = 8
KI = D // 128      # 8 contraction chunks for the projections
QT = 512           # query tile
NQT = S // QT      # 4
KT = 128           # key chunk (contraction tile for PV)
NKT = S // KT      # 16

_cache: dict = {}
PHASE_LOG: list = []
MM_KIND: dict = {}

# ablation switches for perf experiments (leave defaults for production)
_opts = {"attn": True, "outproj": True, "exp": True, "oproj_copy": "dve",
         "mask": True, "norm": True, "proj_copy": "dve", "pool_split": True,
         "lead": 1, "oun": True, "xin_bufs": 3, "psS": 4, "psA": 2, "psO": 2,
         "exp_engine": "act", "oun_engine": "dve", "mask_engine": "dve",
         "fullw": False, "pv_dummy": False}


def _build(repeat: int = 1):
    nc = bacc.Bacc("TRN2", target_bir_lowering=False, debug=False,
                   num_devices=NCORES)

    xqT_d = nc.dram_tensor("xqT", [D, S], f32r, kind="ExternalInput").ap()
    xkT_d = nc.dram_tensor("xkT", [D, S], f32r, kind="ExternalInput").ap()
    xvT_d = nc.dram_tensor("xvT", [D, S], f32r, kind="ExternalInput").ap()
    wqT_d = nc.dram_tensor("wqT", [D, DO], f32r, kind="ExternalInput").ap()
    wkT_d = nc.dram_tensor("wkT", [D, DO], f32r, kind="ExternalInput").ap()
    wvT_d = nc.dram_tensor("wvT", [D, DO], f32r, kind="ExternalInput").ap()
    woT_d = nc.dram_tensor("woT", [DO, D], f32r, kind="ExternalInput").ap()
    cmask_d = nc.dram_tensor("cmask", [128, KT], f32r, kind="ExternalInput").ap()
    out_d = nc.dram_tensor("out", [S, D], f32, kind="ExternalOutput").ap()

    with tile.TileContext(nc) as tc:
        with (
            tc.tile_pool(name="wpool", bufs=1) as wpool,
            tc.tile_pool(name="cpool", bufs=1) as cpool,
            tc.tile_pool(name="persist", bufs=1) as persist,
            tc.tile_pool(name="xin", bufs=_opts["xin_bufs"]) as xin,
            tc.tile_pool(name="ptp", bufs=4) as ptp,
            tc.tile_pool(name="small", bufs=2) as small,
            tc.tile_pool(name="obuf", bufs=2) as obuf,
            tc.tile_pool(name="psS", bufs=_opts["psS"],
                         space=MemorySpace.PSUM) as psS,
            tc.tile_pool(name="psA", bufs=_opts["psA"],
                         space=MemorySpace.PSUM) as psA,
            tc.tile_pool(name="psO", bufs=_opts["psO"],
                         space=MemorySpace.PSUM) as psO,
        ):
            if not _opts["pool_split"]:
                psS = psA
            pools = (nc, wpool, cpool, persist, xin, ptp, small, obuf,
                     psS, psA, psO, xqT_d, xkT_d, xvT_d, wqT_d, wkT_d,
                     wvT_d, woT_d, cmask_d, out_d)
            if repeat > 1:
                with tc.For_i(0, repeat):
                    _emit(*pools)
            else:
                _emit(*pools)

    nc.compile()
    return nc


def _emit(nc, wpool, cpool, persist, xin, ptp, small, obuf, psS, psA, psO,
          xqT_d, xkT_d, xvT_d, wqT_d, wkT_d, wvT_d, woT_d, cmask_d, out_d):
    NT = QT // KT  # 4 key chunks per token block

    # ---- constants / weights ----
    wq_sb = wpool.tile([128, KI, DO], f32r, tag="wq")
    nc.sync.dma_start(wq_sb[:], wqT_d.rearrange("(k p) n -> p k n", p=128))
    wk_sb = wpool.tile([128, KI, DO], f32r, tag="wk")
    nc.sync.dma_start(wk_sb[:], wkT_d.rearrange("(k p) n -> p k n", p=128))
    wv_sb = wpool.tile([128, KI, DO], f32r, tag="wv")
    nc.sync.dma_start(wv_sb[:], wvT_d.rearrange("(k p) n -> p k n", p=128))
    wo_sb = wpool.tile([128, DO // 128, D], f32r, tag="wo")
    nc.sync.dma_start(wo_sb[:], woT_d.rearrange("(k p) n -> p k n", p=128))
    # single triangular mask tile (j >= i), applied to the first 128 cols
    # of the column-restricted diagonal tiles
    tri_sb = cpool.tile([128, KT], f32r, tag="tri")
    nc.sync.dma_start(tri_sb[:], cmask_d)
    ones_f = cpool.tile([1, HD], f32, tag="ones_f")
    nc.vector.memset(ones_f[:], 1.0)
    ones_sb = cpool.tile([1, HD], f32r, tag="ones")
    nc.vector.tensor_copy(ones_sb[:], ones_f[:])
    vones_f = cpool.tile([128, NT * NH], f32, tag="vones_f")
    nc.vector.memset(vones_f[:], 1.0)

    # ---- per-block persistent intermediates ----
    # qT/kT/oT blocks: [256, QT] as [128 parts, 2 chunks, QT]
    #   head j lives in chunk j//2, partitions (j%2)*64 ..+64
    qTt = [persist.tile([128, 2, QT], f32r, tag=f"qT{t}", name=f"qT{t}")
           for t in range(NQT)]
    kTt = [persist.tile([128, 2, QT], f32r, tag=f"kT{t}", name=f"kT{t}")
           for t in range(NQT)]
    oTt = [persist.tile([128, 2, QT], f32r, tag=f"oT{t}", name=f"oT{t}")
           for t in range(NQT)]
    # v blocks, natural layout + ones column: [tokk part, ktc, head, 65]
    vt = [persist.tile([128, NT, NH, HD + 1], f32r, tag=f"v{t}", name=f"v{t}")
          for t in range(NQT)]

    def proj_block(t):
        PHASE_LOG.append((f"proj{t}", nc.next_id()))
        ts = slice(t * QT, (t + 1) * QT)
        xq = xin.tile([128, KI, QT], f32r, tag="xin", name="xq")
        nc.sync.dma_start(
            xq[:], xqT_d[:, ts].rearrange("(k p) n -> p k n", p=128))
        for d in range(2):
            ps = psA.tile([128, QT], f32, tag="ps", name="ps")
            for ki in range(KI):
                nc.tensor.matmul(
                    ps[:], wq_sb[:, ki, d * 128:(d + 1) * 128],
                    xq[:, ki, :], start=(ki == 0), stop=(ki == KI - 1))
            nc.vector.tensor_copy(qTt[t][:, d, :], ps[:])

        xk = xin.tile([128, KI, QT], f32r, tag="xin", name="xk")
        nc.sync.dma_start(
            xk[:], xkT_d[:, ts].rearrange("(k p) n -> p k n", p=128))
        for d in range(2):
            ps = psA.tile([128, QT], f32, tag="ps", name="ps")
            for ki in range(KI):
                nc.tensor.matmul(
                    ps[:], wk_sb[:, ki, d * 128:(d + 1) * 128],
                    xk[:, ki, :], start=(ki == 0), stop=(ki == KI - 1))
            nc.vector.tensor_copy(kTt[t][:, d, :], ps[:])

        xv = xin.tile([128, KI, QT], f32r, tag="xin", name="xv")
        nc.sync.dma_start(
            xv[:], xvT_d[:, ts].rearrange("(k p) n -> p k n", p=128))
        nc.vector.tensor_copy(
            vt[t][:, :, :, HD], vones_f[:].rearrange("p (a b) -> p a b", a=NT))
        for tt in range(NT):
            psv = psA.tile([128, DO], f32, tag="ps", name="psv")
            for ki in range(KI):
                nc.tensor.matmul(
                    psv[:], xv[:, ki, tt * KT:(tt + 1) * KT],
                    wv_sb[:, ki, :], start=(ki == 0), stop=(ki == KI - 1))
            nc.vector.tensor_copy(
                vt[t][:, tt, :, 0:HD],
                psv[:].rearrange("p (h e) -> p h e", h=NH))

    def attn_block(qt):
        PHASE_LOG.append((f"attn{qt}", nc.next_id()))
        # scores/exp lead PV by LEAD steps so the PE never stalls on ACT;
        # accumulators are freed to SBUF right after the last PV; normalize
        # chains run SBUF-local at block end.
        LEAD = _opts["lead"]
        ouns = []
        for j in range(NH if _opts["attn"] else 0):
            poff = (j % 2) * HD
            d = j // 2
            qh = qTt[qt][poff:poff + HD, d, :]
            nkt = (qt + 1) * NT
            pso = psO.tile([HD + 1, QT], f32, tag="pso", name="pso")
            window = {}
            for step in range(nkt + LEAD):
                if step < nkt:
                    kt = step
                    r = kt - qt * NT
                    co = max(r, 0) * KT      # column offset into the q block
                    w = QT - co              # restricted width
                    if _opts["fullw"]:
                        co, w = 0, QT
                    kh = kTt[kt // NT][poff:poff + HD, d,
                                       (kt % NT) * KT:(kt % NT + 1) * KT]
                    pss = psS.tile([128, QT], f32, tag="pss", name="pss")
                    _mi = nc.tensor.matmul(
                        pss[:, 0:w], kh, qh[:, co:QT], start=True, stop=True)
                    try: MM_KIND[_mi.ins.name] = f"score_w{w}"
                    except Exception: pass
                    pt = ptp.tile([128, QT], f32r, tag="pt", name="pt")
                    if _opts["pv_dummy"]:
                        pt = tri_sb  # constant tile; timing-only variant
                    elif not _opts["exp"] or _opts["exp_engine"] == "dve":
                        nc.vector.tensor_copy(pt[:, 0:w], pss[:, 0:w])
                    else:
                        nc.scalar.activation(pt[:, 0:w], pss[:, 0:w], Exp,
                                             scale=0.125)
                    if r >= 0 and _opts["mask"] and not _opts["pv_dummy"]:
                        if _opts["mask_engine"] == "pool":
                            nc.gpsimd.tensor_mul(
                                pt[:, 0:KT], pt[:, 0:KT], tri_sb[:])
                        else:
                            nc.vector.tensor_mul(
                                pt[:, 0:KT], pt[:, 0:KT], tri_sb[:])
                    window[kt] = (co, w, pt)
                if step >= LEAD:
                    kt = step - LEAD
                    co, w, pt = window.pop(kt)
                    if _opts["pv_dummy"]:
                        dw = min(w, KT)
                        _mi = nc.tensor.matmul(
                            pso[:, co:co + dw],
                            vt[kt // NT][:, kt % NT, j, :], pt[:, 0:dw],
                            start=(kt == 0), stop=(kt == nkt - 1))
                    else:
                        _mi = nc.tensor.matmul(
                            pso[:, co:QT], vt[kt // NT][:, kt % NT, j, :],
                            pt[:, 0:w],
                            start=(kt == 0), stop=(kt == nkt - 1))
                    try: MM_KIND[_mi.ins.name] = f"pv_w{w}"
                    except Exception: pass
            if _opts["oun"]:
                # free the PSUM accumulator immediately
                oun = small.tile([HD + 1, QT], f32, tag="oun", name="oun",
                                 bufs=5)
                if _opts["oun_engine"] == "act":
                    nc.scalar.copy(oun[:], pso[:])
                else:
                    nc.vector.tensor_copy(oun[:], pso[:])
                ouns.append((j, oun))
            else:
                ouns.append((j, pso))
        # normalize: columns of oun[0:HD] scaled by 1/rowsum (SBUF-local)
        for j, oun in ouns:
            poff = (j % 2) * HD
            d = j // 2
            if _opts["norm"]:
                recir = small.tile([1, QT], f32r, tag="recir", name="recir")
                with nc.allow_low_precision(reason="f32r normalization scale"):
                    nc.vector.reciprocal(recir[:], oun[HD:HD + 1, :])
                psb = psA.tile([HD, QT], f32, tag="ps", name="psb")
                nc.tensor.matmul(psb[:], ones_sb[:], recir[:],
                                 start=True, stop=True)
                bc = small.tile([HD, QT], f32, tag="bc", name="bc")
                nc.vector.tensor_copy(bc[:], psb[:])
                nc.vector.tensor_mul(
                    oTt[qt][poff:poff + HD, d, :], oun[0:HD, :], bc[:])
            else:
                nc.vector.tensor_copy(
                    oTt[qt][poff:poff + HD, d, :], oun[0:HD, :])

    def oproj_block(t):
        PHASE_LOG.append((f"oproj{t}", nc.next_id()))
        for mtt in range(NT if _opts["outproj"] else 0):
            mt = t * NT + mtt
            for n in range(D // QT):
                ps = psA.tile([128, QT], f32, tag="ps", name="pso2")
                for kc in range(DO // 128):
                    nc.tensor.matmul(
                        ps[:], oTt[t][:, kc, mtt * KT:(mtt + 1) * KT],
                        wo_sb[:, kc, n * QT:(n + 1) * QT],
                        start=(kc == 0), stop=(kc == DO // 128 - 1))
                ob = obuf.tile([128, QT], f32, tag="ob", name="ob")
                if _opts["oproj_copy"] == "act":
                    nc.scalar.copy(ob[:], ps[:])
                else:
                    nc.vector.tensor_copy(ob[:], ps[:])
                nc.sync.dma_start(
                    out_d[mt * 128:(mt + 1) * 128, n * QT:(n + 1) * QT], ob[:])

    # Block-level software pipeline: each block's projections are emitted one
    # block ahead of its attention so the ACT exp stream never waits on
    # just-emitted projections at block boundaries.
    proj_block(0)
    proj_block(1)
    attn_block(0)
    proj_block(2)
    attn_block(1)
    oproj_block(0)
    proj_block(3)
    attn_block(2)
    oproj_block(1)
    attn_block(3)
    oproj_block(2)
    oproj_block(3)


def _round_f32r(x: np.ndarray) -> np.ndarray:
    """Round fp32 to float32r (round-half-up at mantissa bit 12, matching HW)."""
    b = np.ascontiguousarray(x, np.float32).view(np.uint32).astype(np.uint64)
    b = (b + (1 << 11)) & np.uint64(0xFFFFF000)
    return b.astype(np.uint32).view(np.float32)


def _mask_tiles() -> np.ndarray:
    i = np.arange(128)[:, None]
    j = np.arange(KT)[None, :]
    return (j >= i).astype(np.float32)


def make_in_maps(query, key, value, Wq, Wk, Wv, Wo):
    query = np.asarray(query, np.float32)
    key = np.asarray(key, np.float32)
    value = np.asarray(value, np.float32)
    Wq = np.asarray(Wq, np.float32)
    Wk = np.asarray(Wk, np.float32)
    Wv = np.asarray(Wv, np.float32)
    Wo = np.asarray(Wo, np.float32)
    cm = _mask_tiles()
    in_maps = []
    for c in range(NCORES):
        b, hg = divmod(c, NCORES // B)
        sl = slice(hg * DO, (hg + 1) * DO)
        in_maps.append({
            "xqT": _round_f32r(query[b].T),
            "xkT": _round_f32r(key[b].T),
            "xvT": _round_f32r(value[b].T),
            "wqT": _round_f32r(Wq[sl].T),
            "wkT": _round_f32r(Wk[sl].T),
            "wvT": _round_f32r(Wv[sl].T),
            "woT": _round_f32r(Wo[:, sl].T),
            "cmask": cm,
        })
    return in_maps


def kernel(query, key, value, freqs_complex_form, mask, Wq, Wk, Wv, Wo):
    if "nc" not in _cache:
        _cache["nc"] = _build()
    nc = _cache["nc"]
    in_maps = make_in_maps(query, key, value, Wq, Wk, Wv, Wo)
    res = run_bass_kernel_spmd(nc, in_maps, list(range(NCORES)))
    parts = [res.results[c]["out"] for c in range(NCORES)]
    npg = NCORES // B
    return np.stack(
        [np.sum(parts[b * npg:(b + 1) * npg], axis=0) for b in range(B)]
    ).astype(np.float32)



# revision 12
# speedup vs baseline: 1.4579x; 1.4579x over previous
"""Multi-head attention (B=2, S=2048, D=1024, H=16, causal) on 8 TRN2 NeuronCores.

Sharding: core c -> (batch b = c//4, head-group hg = c%4). Each core:
  - projects its batch's query/key/value against a 256-row slice of Wq/Wk/Wv
    (4 heads of 64 dims),
  - runs causal attention for those 4 heads (scores computed transposed,
    exp on ACT with fused 1/8 scale, row-sums via a ones-column in V),
  - multiplies by the matching 256-column slice of Wo -> partial [2048, 1024].
Host sums the 4 partials per batch (the tensor-parallel all-reduce) and stacks.

Performance structure: the TRN2 PE ramps to 2.4 GHz only after ~3us of
continuous busy time and drops to 1.2 GHz after any idle gap, and the per-chunk
exp stream on ACT is slower than the score+PV matmuls at full clock. So all
projection / output-projection matmul work is emitted as "filler" units
interleaved into the attention stream: whenever attention would wait on ACT,
the PE has independent proj/oproj work queued behind it. Everything runs in
bf16 (full-rate matmuls at any width, half the DMA/LDWEIGHTS traffic);
accumulation stays fp32 in PSUM. Normalization uses a fast DVE reciprocal and
a pair-packed selector matmul to broadcast the per-token scales.
"""

import sys

for _p in ("/opt/trn_rl_repo", "/root/.axon_site/_ro/trn_rl_repo"):
    if _p not in sys.path:
        sys.path.append(_p)

from collections import deque

import numpy as np
import ml_dtypes

import concourse.bacc as bacc
import concourse.tile as tile
import concourse.mybir as mybir
from concourse.bass import MemorySpace
from concourse.bass_utils import run_bass_kernel_spmd

f32 = mybir.dt.float32
f32r = mybir.dt.float32r
bf16 = mybir.dt.bfloat16
Exp = mybir.ActivationFunctionType.Exp

B, S, D, H = 2, 2048, 1024, 16
HD = 64            # head dim
NH = 4             # heads per core
DO = NH * HD       # 256 projection out-dims per core
NCORES = 8
KI = D // 128      # 8 contraction chunks for the projections
QT = 512           # query tile
NQT = S // QT      # 4
KT = 128           # key chunk (contraction tile for PV)
NKT = S // KT      # 16
NT = QT // KT      # 4 key chunks per query block

_cache: dict = {}

_opts = {"lead": 1, "ratio": 0.5, "reserve": 4}


def _build():
    nc = bacc.Bacc("TRN2", target_bir_lowering=False, debug=False,
                   num_devices=NCORES)

    xqT_d = nc.dram_tensor("xqT", [D, S], bf16, kind="ExternalInput").ap()
    xkT_d = nc.dram_tensor("xkT", [D, S], bf16, kind="ExternalInput").ap()
    xvT_d = nc.dram_tensor("xvT", [D, S], bf16, kind="ExternalInput").ap()
    wqT_d = nc.dram_tensor("wqT", [D, DO], bf16, kind="ExternalInput").ap()
    wkT_d = nc.dram_tensor("wkT", [D, DO], bf16, kind="ExternalInput").ap()
    wvT_d = nc.dram_tensor("wvT", [D, DO], bf16, kind="ExternalInput").ap()
    woT_d = nc.dram_tensor("woT", [DO, D], bf16, kind="ExternalInput").ap()
    cmask_d = nc.dram_tensor("cmask", [128, KT], bf16, kind="ExternalInput").ap()
    selr_d = nc.dram_tensor("selr", [128, 2 * HD * 2], f32r,
                            kind="ExternalInput").ap()
    out_d = nc.dram_tensor("out", [S, D], f32, kind="ExternalOutput").ap()

    with tile.TileContext(nc) as tc:
        with (
            tc.tile_pool(name="wpool", bufs=1) as wpool,
            tc.tile_pool(name="cpool", bufs=1) as cpool,
            tc.tile_pool(name="persist", bufs=1) as persist,
            tc.tile_pool(name="xin", bufs=12) as xin,
            tc.tile_pool(name="ptp", bufs=4) as ptp,
            tc.tile_pool(name="small", bufs=2) as small,
            tc.tile_pool(name="obuf", bufs=2) as obuf,
            tc.tile_pool(name="psS", bufs=2, space=MemorySpace.PSUM) as psS,
            tc.tile_pool(name="psA", bufs=3, space=MemorySpace.PSUM) as psA,
            tc.tile_pool(name="psO", bufs=2, space=MemorySpace.PSUM) as psO,
            tc.tile_pool(name="psN", bufs=1, space=MemorySpace.PSUM) as psN,
        ):
            _emit(nc, wpool, cpool, persist, xin, ptp, small, obuf,
                  psS, psA, psO, psN, xqT_d, xkT_d, xvT_d, wqT_d, wkT_d,
                  wvT_d, woT_d, cmask_d, selr_d, out_d)

    nc.compile()
    return nc


def _emit(nc, wpool, cpool, persist, xin, ptp, small, obuf, psS, psA, psO, psN,
          xqT_d, xkT_d, xvT_d, wqT_d, wkT_d, wvT_d, woT_d, cmask_d, selr_d,
          out_d):
    # ---- weights + constants; x DMAs issued up front (12 tiles coexist) ----
    wq_sb = wpool.tile([128, KI, DO], bf16, tag="wq")
    nc.sync.dma_start(wq_sb[:], wqT_d.rearrange("(k p) n -> p k n", p=128))
    xq_t, xk_t, xv_t = [], [], []

    def dma_x(lst, dram, t, name):
        ts = slice(t * QT, (t + 1) * QT)
        tl = xin.tile([128, KI, QT], bf16, tag="xin", name=f"{name}{t}")
        nc.sync.dma_start(tl[:], dram[:, ts].rearrange("(k p) n -> p k n", p=128))
        lst.append(tl)

    dma_x(xq_t, xqT_d, 0, "xq")
    wk_sb = wpool.tile([128, KI, DO], bf16, tag="wk")
    nc.sync.dma_start(wk_sb[:], wkT_d.rearrange("(k p) n -> p k n", p=128))
    dma_x(xk_t, xkT_d, 0, "xk")
    wv_sb = wpool.tile([128, KI, DO], bf16, tag="wv")
    nc.sync.dma_start(wv_sb[:], wvT_d.rearrange("(k p) n -> p k n", p=128))
    dma_x(xv_t, xvT_d, 0, "xv")
    wo_sb = wpool.tile([128, DO // 128, D], bf16, tag="wo")
    nc.sync.dma_start(wo_sb[:], woT_d.rearrange("(k p) n -> p k n", p=128))
    tri_sb = cpool.tile([128, KT], bf16, tag="tri")
    nc.sync.dma_start(tri_sb[:], cmask_d)
    for t in range(1, NQT):
        dma_x(xq_t, xqT_d, t, "xq")
        dma_x(xk_t, xkT_d, t, "xk")
        dma_x(xv_t, xvT_d, t, "xv")

    # selector for the pair-packed reciprocal broadcast (host-built since
    # engine writes must start at 32-aligned partitions):
    # sel[p, c] = 1 iff p == 32 * (c // HD); rowsums are parked at
    # partitions {0,32,64,96} of the rs tile.
    sel = cpool.tile([128, 2 * HD * 2], f32r, tag="sel")
    nc.sync.dma_start(sel[:], selr_d)
    vones_f = cpool.tile([128, NT * NH], f32, tag="vones_f")
    nc.gpsimd.memset(vones_f[:], 1.0)
    vones = cpool.tile([128, NT * NH], bf16, tag="vones")
    nc.vector.tensor_copy(vones[:], vones_f[:])

    # ---- per-block persistent intermediates ----
    # qT/kT/oT blocks: [256, QT] as [128 parts, 2 chunks, QT]
    #   head j lives in chunk j//2, partitions (j%2)*64 ..+64
    qTt = [persist.tile([128, 2, QT], bf16, tag=f"qT{t}", name=f"qT{t}")
           for t in range(NQT)]
    kTt = [persist.tile([128, 2, QT], bf16, tag=f"kT{t}", name=f"kT{t}")
           for t in range(NQT)]
    oTt = [persist.tile([128, 2, QT], bf16, tag=f"oT{t}", name=f"oT{t}")
           for t in range(NQT)]
    # v blocks, natural layout + ones column: [tokk part, ktc, head, 65]
    vt = [persist.tile([128, NT, NH, HD + 1], bf16, tag=f"v{t}", name=f"v{t}")
          for t in range(NQT)]

    # ---- filler units: (cycles, tag, closure) drained into the PE stream ----
    fillers = deque()
    state = {"deficit": 0.0}
    RATIO = _opts["ratio"]

    def drain(cycles, reserve=0):
        state["deficit"] += cycles * RATIO
        while (len(fillers) > reserve
               and state["deficit"] >= fillers[0][0]):
            cyc, _tag, fn = fillers.popleft()
            fn()
            state["deficit"] -= cyc

    def force_units(pred):
        while fillers and pred(fillers[0][1]):
            _cyc, _tag, fn = fillers.popleft()
            fn()

    def proj_units(t):
        units = []
        for d in range(2):
            def qunit(t=t, d=d):
                ps = psA.tile([128, QT], f32, tag="ps", name="psq")
                for ki in range(KI):
                    nc.tensor.matmul(
                        ps[:], wq_sb[:, ki, d * 128:(d + 1) * 128],
                        xq_t[t][:, ki, :], start=(ki == 0), stop=(ki == KI - 1))
                nc.vector.tensor_copy(qTt[t][:, d, :], ps[:])
            units.append((8 * QT, ("proj", t), qunit))
        for d in range(2):
            def kunit(t=t, d=d):
                ps = psA.tile([128, QT], f32, tag="ps", name="psk")
                for ki in range(KI):
                    nc.tensor.matmul(
                        ps[:], wk_sb[:, ki, d * 128:(d + 1) * 128],
                        xk_t[t][:, ki, :], start=(ki == 0), stop=(ki == KI - 1))
                nc.vector.tensor_copy(kTt[t][:, d, :], ps[:])
            units.append((8 * QT, ("proj", t), kunit))
        for tt in range(NT):
            def vunit(t=t, tt=tt):
                if tt == 0:
                    nc.gpsimd.tensor_copy(
                        vt[t][:, :, :, HD],
                        vones[:].rearrange("p (a b) -> p a b", a=NT))
                psv = psA.tile([128, DO], f32, tag="ps", name="psv")
                for ki in range(KI):
                    nc.tensor.matmul(
                        psv[:], xv_t[t][:, ki, tt * KT:(tt + 1) * KT],
                        wv_sb[:, ki, :], start=(ki == 0), stop=(ki == KI - 1))
                nc.vector.tensor_copy(
                    vt[t][:, tt, :, 0:HD],
                    psv[:].rearrange("p (h e) -> p h e", h=NH))
            units.append((8 * DO, ("proj", t), vunit))
        return units

    def oproj_units(t):
        units = []
        for mtt in range(NT):
            for n in range(D // QT):
                def ounit(t=t, mtt=mtt, n=n):
                    mt = t * NT + mtt
                    ps = psA.tile([128, QT], f32, tag="ps", name="pso2")
                    for kc in range(DO // 128):
                        nc.tensor.matmul(
                            ps[:], oTt[t][:, kc, mtt * KT:(mtt + 1) * KT],
                            wo_sb[:, kc, n * QT:(n + 1) * QT],
                            start=(kc == 0), stop=(kc == DO // 128 - 1))
                    ob = obuf.tile([128, QT], f32, tag="ob", name="ob")
                    nc.vector.tensor_copy(ob[:], ps[:])
                    nc.sync.dma_start(
                        out_d[mt * 128:(mt + 1) * 128, n * QT:(n + 1) * QT],
                        ob[:])
                units.append((2 * QT, ("oproj", t), ounit))
        return units

    # ---- attention: scores -> exp (ACT) -> mask (Pool) -> PV, with the
    # filler stream keeping the PE dense; normalization is deferred into the
    # next block's stream so its serial chain hides behind attention work ----
    def attn_block(qt, reserve=0):
        LEAD = _opts["lead"]
        nkt = (qt + 1) * NT
        ouns = []
        for j in range(NH):
            poff = (j % 2) * HD
            d = j // 2
            qh = qTt[qt][poff:poff + HD, d, :]
            pso = psO.tile([HD + 1, QT], f32, tag="pso", name="pso")
            window = {}
            for step in range(nkt + LEAD):
                if step < nkt:
                    kt = step
                    r = kt - qt * NT
                    co = max(r, 0) * KT
                    w = QT - co
                    kh = kTt[kt // NT][poff:poff + HD, d,
                                       (kt % NT) * KT:(kt % NT + 1) * KT]
                    pss = psS.tile([128, QT], f32, tag="pss", name="pss")
                    nc.tensor.matmul(pss[:, 0:w], kh, qh[:, co:QT],
                                     start=True, stop=True)
                    pt = ptp.tile([128, QT], bf16, tag="pt", name="pt")
                    nc.scalar.activation(pt[:, 0:w], pss[:, 0:w], Exp,
                                         scale=0.125)
                    if r >= 0:
                        nc.gpsimd.tensor_mul(pt[:, 0:KT], pt[:, 0:KT],
                                             tri_sb[:])
                    window[kt] = (co, w, pt)
                    drain(w, reserve)
                if step >= LEAD:
                    kt = step - LEAD
                    co, w, pt = window.pop(kt)
                    nc.tensor.matmul(
                        pso[:, co:QT], vt[kt // NT][:, kt % NT, j, :],
                        pt[:, 0:w], start=(kt == 0), stop=(kt == nkt - 1))
                    drain(w, reserve)
            # free the PSUM accumulator (DVE: keeps ACT's exp stream dense).
            # bufs=9: norm(qt) is emitted after attn(qt+1), so two blocks'
            # worth of oun tiles (8) are alive at once.
            oun = small.tile([HD + 1, QT], f32, tag="oun", name="oun", bufs=9)
            nc.vector.tensor_copy(oun[:], pso[:])
            ouns.append(oun)
        # rowsums -> 1/x on DVE (fast approx; sums are in [1, ~12000]).
        # Unused partitions hold 1.0 so the reciprocal stays finite; the
        # selector matmul zeroes them out anyway.
        rs = small.tile([128, QT], f32, tag="rs", name="rs")
        nc.gpsimd.memset(rs[:], 1.0)
        for j in range(NH):
            nc.gpsimd.tensor_copy(rs[32 * j:32 * j + 1, :],
                                  ouns[j][HD:HD + 1, :])
        rec_f = small.tile([128, QT], f32, tag="rec", name="rec")
        nc.vector.reciprocal_approx_fast(rec_f[:], rs[:])
        rec_r = small.tile([128, QT], f32r, tag="recr", name="recr")
        with nc.allow_low_precision(reason="f32r normalization scale"):
            nc.vector.tensor_copy(rec_r[:], rec_f[:])

        def norm(qt=qt, ouns=ouns, rec_r=rec_r):
            for dd in range(2):
                psb = psN.tile([128, QT], f32, tag="psb", name="psb")
                nc.tensor.matmul(psb[:], sel[:, dd * 128:(dd + 1) * 128],
                                 rec_r[:], start=True, stop=True)
                for pp in range(2):
                    nc.vector.tensor_mul(
                        oTt[qt][pp * HD:(pp + 1) * HD, dd, :],
                        ouns[2 * dd + pp][0:HD, :], psb[pp * HD:(pp + 1) * HD, :])
                drain(QT, 0)
        return norm

    # ---- schedule ----
    for u in proj_units(0):
        u[2]()
    fillers.extend(proj_units(1))
    fillers.extend(proj_units(2))
    fillers.extend(proj_units(3))

    norm_prev = attn_block(0)

    force_units(lambda tag: tag == ("proj", 1))
    norm_prev2 = attn_block(1)
    norm_prev()                       # norm0: its recip chain ran during attn1
    fillers.extend(oproj_units(0))
    norm_prev = norm_prev2

    force_units(lambda tag: tag == ("proj", 2))
    norm_prev2 = attn_block(2)
    norm_prev()
    fillers.extend(oproj_units(1))
    norm_prev = norm_prev2

    force_units(lambda tag: tag == ("proj", 3))
    norm_prev2 = attn_block(3, reserve=_opts["reserve"])
    norm_prev()
    fillers.extend(oproj_units(2))
    # drain everything except a small reserve to cover norm3's serial chain
    while len(fillers) > _opts["reserve"]:
        _cyc, _tag, fn = fillers.popleft()
        fn()
    norm_prev2()
    fillers.extend(oproj_units(3))
    force_units(lambda tag: True)


def _mask_tiles() -> np.ndarray:
    i = np.arange(128)[:, None]
    j = np.arange(KT)[None, :]
    return (j >= i).astype(ml_dtypes.bfloat16)


def _sel_mat() -> np.ndarray:
    p = np.arange(128)[:, None]
    c = np.arange(2 * HD * 2)[None, :]
    return (p == 32 * (c // HD)).astype(np.float32)


def make_in_maps(query, key, value, Wq, Wk, Wv, Wo):
    bf = ml_dtypes.bfloat16
    query = np.asarray(query, np.float32)
    key = np.asarray(key, np.float32)
    value = np.asarray(value, np.float32)
    Wq = np.asarray(Wq, np.float32)
    Wk = np.asarray(Wk, np.float32)
    Wv = np.asarray(Wv, np.float32)
    Wo = np.asarray(Wo, np.float32)
    cm = _mask_tiles()
    in_maps = []
    for c in range(NCORES):
        b, hg = divmod(c, NCORES // B)
        sl = slice(hg * DO, (hg + 1) * DO)
        in_maps.append({
            "xqT": np.ascontiguousarray(query[b].T).astype(bf),
            "xkT": np.ascontiguousarray(key[b].T).astype(bf),
            "xvT": np.ascontiguousarray(value[b].T).astype(bf),
            "wqT": np.ascontiguousarray(Wq[sl].T).astype(bf),
            "wkT": np.ascontiguousarray(Wk[sl].T).astype(bf),
            "wvT": np.ascontiguousarray(Wv[sl].T).astype(bf),
            "woT": np.ascontiguousarray(Wo[:, sl].T).astype(bf),
            "cmask": cm,
            "selr": _sel_mat(),
        })
    return in_maps


def kernel(query, key, value, freqs_complex_form, mask, Wq, Wk, Wv, Wo):
    if "nc" not in _cache:
        _cache["nc"] = _build()
    nc = _cache["nc"]
    in_maps = make_in_maps(query, key, value, Wq, Wk, Wv, Wo)
    res = run_bass_kernel_spmd(nc, in_maps, list(range(NCORES)))
    parts = [res.results[c]["out"] for c in range(NCORES)]
    npg = NCORES // B
    return np.stack(
        [np.sum(parts[b * npg:(b + 1) * npg], axis=0) for b in range(B)]
    ).astype(np.float32)
